# revision 9
# baseline (speedup 1.0000x reference)
"""Trainium2 Bass kernel for the LSTM trajectory decoder.

Algorithm notes (host-side algebraic folding):
  The reference per-step computation is
      x_t   = W_se @ [rel_{t-1}; spd_t] + b_se          (spatial embedding)
      gates = W_ih @ x_t + W_hh @ h_t + b
      c,h   = LSTM cell update
      rel_t = W_hp @ h_{t+1} + b_hp
  Since rel_{t-1} = W_hp @ h_t + b_hp is linear in h_t, the whole embedding
  path folds into the recurrent weights:
      W_hh_eff = W_hh + (W_ih @ W_se[:, :2]) @ W_hp
  leaving only a rank-2 exogenous input per step: [1 (bias); spd_t].
  All sigmoids/tanhs are computed as sigmoids via tanh(x) = 2*sigmoid(2x) - 1,
  with the 2x/0.5x factors pre-folded into weights; the device keeps
  hhat = h/2 as its hidden state.

Device layout (per core, batch 4096 of 32768):
  states [128 hidden partitions, batch free], gates via PE matmuls into PSUM
  (4 banks/chunk, double buffered), one fused sigmoid ACT op per chunk over
  all 4 gates, cell update via scalar_tensor_tensor ops on DVE/GPSIMD,
  position readout via transposed matmuls (hhat stationary) packed into a
  [128, 64] PSUM tile.
"""

import numpy as np

B, E, H, T = 32768, 64, 128, 12
NCORES = 8
BC = B // NCORES          # 4096 batch per core
NG = 4                    # elementwise groups per step
GW = BC // NG             # 1024 group width
CW = 512                  # matmul chunk width (fp32 moving-operand max)
NCHUNK = BC // CW         # 8

_STATE = {}


def _import_concourse():
    import sys
    for p in ('/opt/trn_rl_repo', '/root/.axon_site/_ro/trn_rl_repo'):
        if p not in sys.path:
            sys.path.append(p)


def _fold_weights(W_se, b_se, W_ih, W_hh, b_ih, b_hh, W_hp, b_hp):
    """Host-side weight folding in float64; returns device weight arrays."""
    f8 = np.float64
    W_se, b_se, W_ih, W_hh = W_se.astype(f8), b_se.astype(f8), W_ih.astype(f8), W_hh.astype(f8)
    b_ih, b_hh, W_hp, b_hp = b_ih.astype(f8), b_hh.astype(f8), W_hp.astype(f8), b_hp.astype(f8)

    WU = W_ih @ W_se                  # [4H, 3]
    R = WU[:, 0:2]
    w_spd = WU[:, 2]
    b0 = b_ih + b_hh + W_ih @ b_se
    W_hh_eff = W_hh + R @ W_hp
    b_eff = b0 + R @ b_hp

    def reorder(w):                   # torch [i,f,g,o] -> device [i,f,o,g]
        i, f, g, o = np.split(w, 4, axis=0)
        return np.concatenate([i, f, o, g], axis=0)

    gscale = np.ones((4 * H, 1))
    gscale[3 * H:] = 2.0              # g block: tanh(x) = 2 sig(2x) - 1

    lhsT0_hh = (reorder(W_hh) * gscale * 2.0).T.astype(np.float32)       # [H, 4H]
    lhsT_hh = (reorder(W_hh_eff) * gscale * 2.0).T.astype(np.float32)    # [H, 4H]
    lhsT_u0 = (np.stack([reorder(b0), reorder(R[:, 0]), reorder(R[:, 1]),
                         reorder(w_spd)], axis=1) * gscale).T.astype(np.float32)  # [4, 4H]
    lhsT_sb = (np.stack([reorder(b_eff), reorder(w_spd)], axis=1) * gscale).T.astype(np.float32)  # [2, 4H]
    W_hpT2 = np.ascontiguousarray((2.0 * W_hp).T).astype(np.float32)     # [H, 2]
    return dict(w_hh0=lhsT0_hh, w_hh=lhsT_hh, w_u0=lhsT_u0, w_sb=lhsT_sb, w_hp=W_hpT2)


def _build_program():
    _import_concourse()
    import concourse.bacc as bacc
    import concourse.tile as tile
    from concourse import mybir

    fp32 = mybir.dt.float32
    AF = mybir.ActivationFunctionType
    OP = mybir.AluOpType

    nc = bacc.Bacc("TRN2", target_bir_lowering=False, debug=False)

    # DRAM I/O (per-core shapes)
    d_h0 = nc.dram_tensor("h0t", [H, BC], fp32, kind="ExternalInput")       # hhat0 = h0/2, [H, batch]
    d_c0 = nc.dram_tensor("c0t", [H, BC], fp32, kind="ExternalInput")
    d_spd = nc.dram_tensor("spdones", [2 * T, BC], fp32, kind="ExternalInput")  # rows 2t=ones, 2t+1=spd_t
    d_u0 = nc.dram_tensor("u0", [4, BC], fp32, kind="ExternalInput")        # [ones; relx; rely; spd0]
    d_whh0 = nc.dram_tensor("w_hh0", [H, 4 * H], fp32, kind="ExternalInput")
    d_whh = nc.dram_tensor("w_hh", [H, 4 * H], fp32, kind="ExternalInput")
    d_wu0 = nc.dram_tensor("w_u0", [4, 4 * H], fp32, kind="ExternalInput")
    d_wsb = nc.dram_tensor("w_sb", [2, 4 * H], fp32, kind="ExternalInput")
    d_whp = nc.dram_tensor("w_hp", [H, 2], fp32, kind="ExternalInput")
    d_rels = nc.dram_tensor("rels", [T, 128, 2 * (BC // 128)], fp32, kind="ExternalOutput")
    d_hout = nc.dram_tensor("hout", [H, BC], fp32, kind="ExternalOutput")

    with tile.TileContext(nc) as tc:
        with (
            tc.tile_pool(name="singles", bufs=1) as singles,
            tc.tile_pool(name="tmps", bufs=2) as tmps,
            tc.tile_pool(name="relsb", bufs=2) as relsb,
            tc.tile_pool(name="psum", bufs=2, space="PSUM") as psum,
        ):
            # --- load constants / weights ---
            whh0 = singles.tile([H, 4 * H], fp32)
            whh = singles.tile([H, 4 * H], fp32)
            wu0 = singles.tile([4, 4 * H], fp32)
            wsb = singles.tile([2, 4 * H], fp32)
            whp = singles.tile([H, 2], fp32)
            u0 = singles.tile([4, BC], fp32)
            nc.sync.dma_start(whh0[:], d_whh0[:])
            nc.sync.dma_start(whh[:], d_whh[:])
            nc.sync.dma_start(wu0[:], d_wu0[:])
            nc.sync.dma_start(wsb[:], d_wsb[:])
            nc.sync.dma_start(whp[:], d_whp[:])
            nc.sync.dma_start(u0[:], d_u0[:])

            # --- state tiles: hidden (two parities) and cell, per group ---
            Hh = [[singles.tile([H, 2, CW], fp32, tag=f"h{p}g{g}", name=f"h{p}g{g}") for g in range(NG)]
                  for p in range(2)]
            C = [singles.tile([H, 2, CW], fp32, tag=f"c{g}", name=f"c{g}") for g in range(NG)]
            SIG = [singles.tile([H, 2, 4, CW], fp32, tag=f"sig{g}", name=f"sig{g}") for g in range(NG)]
            for g in range(NG):
                nc.sync.dma_start(Hh[0][g][:], d_h0[:, g * GW:(g + 1) * GW].rearrange("h (a b) -> h a b", b=CW))
                nc.sync.dma_start(C[g][:], d_c0[:, g * GW:(g + 1) * GW].rearrange("h (a b) -> h a b", b=CW))

            # --- the 12 recurrent steps ---
            for t in range(T):
                Hcur, Hnew = Hh[t % 2], Hh[(t + 1) % 2]
                wh = whh0 if t == 0 else whh
                if t > 0:
                    # stage this step's [ones; spd_t] rows at base partition 0
                    spd_t = tmps.tile([2, BC], fp32, tag="spd", name=f"spd{t}")
                    nc.sync.dma_start(spd_t[:], d_spd[2 * t:2 * t + 2, :])
                # gates: 8 chunks of 512 batch columns
                for ch in range(NCHUNK):
                    g, j = ch // 2, ch % 2
                    rhs_h = Hcur[g][:, j, :]                       # [H, 512]
                    if t == 0:
                        rhs_u = u0[:, ch * CW:(ch + 1) * CW]       # [4, 512]
                        wu = wu0
                    else:
                        rhs_u = spd_t[:, ch * CW:(ch + 1) * CW]    # [ones; spd_t]
                        wu = wsb
                    gt = psum.tile([128, 4, CW], fp32, tag="gates")
                    for gi in range(4):
                        nc.tensor.matmul(gt[:, gi, :], wh[:, gi * 128:(gi + 1) * 128],
                                         rhs_h, start=True, stop=False)
                        nc.tensor.matmul(gt[:, gi, :], wu[:, gi * 128:(gi + 1) * 128],
                                         rhs_u, start=False, stop=True)
                    # fused sigmoid over all 4 gates of this chunk
                    nc.scalar.activation(SIG[g][:, j, :, :], gt[:, :, :], AF.Sigmoid)

                # cell update per group of 1024
                for g in range(NG):
                    si = SIG[g][:, :, 0, :]
                    sf = SIG[g][:, :, 1, :]
                    so = SIG[g][:, :, 2, :]
                    sg2 = SIG[g][:, :, 3, :]
                    u_t = tmps.tile([H, 2, CW], fp32, tag="u")
                    v_t = tmps.tile([H, 2, CW], fp32, tag="v")
                    w2c = tmps.tile([H, 2, CW], fp32, tag="w")
                    # u = (sig(2g) - 0.5) * sig(i)   [= tanh(g)*sig(i)/2]
                    nc.vector.scalar_tensor_tensor(u_t[:], sg2, 0.5, si,
                                                   op0=OP.subtract, op1=OP.mult)
                    # v = sig(f) * c   (gpsimd: keeps DVE free for the STT ops)
                    nc.gpsimd.tensor_tensor(v_t[:], sf, C[g][:], op=OP.mult)
                    # c' = 2u + v
                    nc.vector.scalar_tensor_tensor(C[g][:], u_t[:], 2.0, v_t[:],
                                                   op0=OP.mult, op1=OP.add)
                    # w = sig(2 c')
                    nc.scalar.activation(w2c[:], C[g][:], AF.Sigmoid, scale=2.0)
                    # hhat' = (w - 0.5) * sig(o)   [= tanh(c')*sig(o)/2]
                    nc.vector.scalar_tensor_tensor(Hnew[g][:], w2c[:], 0.5, so,
                                                   op0=OP.subtract, op1=OP.mult)

                # rel readout: rel_t = (2 W_hp) @ hhat_{t+1}, transposed layout
                rp = psum.tile([128, 2 * (BC // 128)], fp32, tag="gates")
                for k in range(BC // 128):
                    g, r = k // (GW // 128), k % (GW // 128)
                    j, kk = r // (CW // 128), r % (CW // 128)
                    lhsT = Hnew[g][:, j, kk * 128:(kk + 1) * 128]  # [H, 128]
                    nc.tensor.matmul(rp[:, 2 * k:2 * k + 2], lhsT, whp[:],
                                     start=True, stop=True)
                rsb = relsb.tile([128, 2 * (BC // 128)], fp32, tag="rsb")
                nc.scalar.copy(rsb[:], rp[:])
                nc.sync.dma_start(d_rels[t], rsb[:])

            # final hidden state out (hhat; host multiplies by 2)
            Hfin = Hh[T % 2]
            for g in range(NG):
                nc.sync.dma_start(d_hout[:, g * GW:(g + 1) * GW].rearrange("h (a b) -> h a b", b=CW),
                                  Hfin[g][:])

    nc.compile()
    return nc


def _build_runner(nc):
    """Cached jitted SPMD runner (mirrors bass2jax.run_bass_via_pjrt)."""
    _import_concourse()
    import jax
    from jax.sharding import Mesh, PartitionSpec
    from jax.experimental.shard_map import shard_map
    from concourse import bass2jax, mybir

    bass2jax.install_neuronx_cc_hook()

    partition_name = nc.partition_id_tensor.name if nc.partition_id_tensor else None

    in_names, out_names, out_avals, zero_shapes = [], [], [], []
    for alloc in nc.m.functions[0].allocations:
        if not isinstance(alloc, mybir.MemoryLocationSet):
            continue
        name = alloc.memorylocations[0].name
        if alloc.kind == "ExternalInput":
            if name != partition_name:
                in_names.append(name)
        elif alloc.kind == "ExternalOutput":
            out_names.append(name)
            shape = tuple(alloc.tensor_shape)
            dtype = mybir.dt.np(alloc.dtype)
            out_avals.append(jax.core.ShapedArray(shape, dtype))
            zero_shapes.append((shape, dtype))
    n_params = len(in_names)
    n_outs = len(out_names)
    all_names = in_names + out_names
    if partition_name is not None:
        all_names = all_names + [partition_name]
    donate = tuple(range(n_params, n_params + n_outs))

    def _body(*args):
        operands = list(args)
        if partition_name is not None:
            operands.append(bass2jax.partition_id_tensor())
        outs = bass2jax._bass_exec_p.bind(
            *operands,
            out_avals=tuple(out_avals),
            in_names=tuple(all_names),
            out_names=tuple(out_names),
            lowering_input_output_aliases=(),
            sim_require_finite=False,
            sim_require_nnan=False,
            nc=nc,
        )
        return tuple(outs)

    devices = jax.devices()[:NCORES]
    mesh = Mesh(np.asarray(devices), ("core",))
    in_specs = (PartitionSpec("core"),) * (n_params + n_outs)
    out_specs = (PartitionSpec("core"),) * n_outs
    sharded = jax.jit(
        shard_map(_body, mesh=mesh, in_specs=in_specs, out_specs=out_specs,
                  check_rep=False),
        donate_argnums=donate, keep_unused=True,
    )

    def run(in_maps):
        concat_in = [np.concatenate([m[name] for m in in_maps], axis=0)
                     for name in in_names]
        concat_zeros = [np.zeros((NCORES * s[0], *s[1:]), d) for s, d in zero_shapes]
        out_arrs = sharded(*concat_in, *concat_zeros)
        return {
            name: np.asarray(out_arrs[i]).reshape(NCORES, *zero_shapes[i][0])
            for i, name in enumerate(out_names)
        }

    return run


def _get_runner():
    if "run" not in _STATE:
        nc = _build_program()
        _STATE["run"] = _build_runner(nc)
    return _STATE["run"]


def kernel(last_pos, last_pos_rel, h0, c0, pred_ped_speed,
           W_se, b_se, W_ih, W_hh, b_ih, b_hh, W_hp, b_hp):
    last_pos_rel = np.asarray(last_pos_rel, np.float32)
    h0 = np.asarray(h0, np.float32)
    c0 = np.asarray(c0, np.float32)
    pred_ped_speed = np.asarray(pred_ped_speed, np.float32)

    wd = _fold_weights(np.asarray(W_se), np.asarray(b_se), np.asarray(W_ih),
                       np.asarray(W_hh), np.asarray(b_ih), np.asarray(b_hh),
                       np.asarray(W_hp), np.asarray(b_hp))

    # shard + transpose inputs on host
    h0t = np.ascontiguousarray(h0[0].T) * 0.5          # [H, B], hhat
    c0t = np.ascontiguousarray(c0[0].T)                # [H, B]
    relT = last_pos_rel.T                              # [2, B]
    spd = pred_ped_speed[:, :, 0]                      # [T, B]

    spdones = np.ones((2 * T, B), np.float32)
    spdones[1::2, :] = spd
    u0 = np.ones((4, B), np.float32)
    u0[1:3] = relT
    u0[3] = spd[0]

    in_maps = []
    for c in range(NCORES):
        s = slice(c * BC, (c + 1) * BC)
        m = dict(h0t=np.ascontiguousarray(h0t[:, s]),
                 c0t=np.ascontiguousarray(c0t[:, s]),
                 spdones=np.ascontiguousarray(spdones[:, s]),
                 u0=np.ascontiguousarray(u0[:, s]))
        m.update(wd)
        in_maps.append(m)

    run = _get_runner()
    outs = run(in_maps)

    # rels: [core][T, 128, 32*2] -> [T, B, 2]
    r = outs["rels"]                                   # [NCORES, T, 128, 64]
    r = r.reshape(NCORES, T, 128, BC // 128, 2)
    r = r.transpose(1, 0, 3, 2, 4).reshape(T, B, 2)
    rels = r + np.asarray(b_hp, np.float32)[None, None, :]

    h = outs["hout"]                                   # [NCORES, H, BC]
    h = (2.0 * h).transpose(0, 2, 1).reshape(1, B, H)
    return np.ascontiguousarray(rels.astype(np.float32)), np.ascontiguousarray(h.astype(np.float32))


# revision 27
# speedup vs baseline: 4092.0590x; 4092.0590x over previous
"""Trainium2 Bass kernel for the LSTM trajectory decoder.

Algorithm notes (host-side algebraic folding):
  The reference per-step computation is
      x_t   = W_se @ [rel_{t-1}; spd_t] + b_se          (spatial embedding)
      gates = W_ih @ x_t + W_hh @ h_t + b
      c,h   = LSTM cell update
      rel_t = W_hp @ h_{t+1} + b_hp
  Since rel_{t-1} = W_hp @ h_t + b_hp is linear in h_t, the whole embedding
  path folds into the recurrent weights:
      W_hh_eff = W_hh + (W_ih @ W_se[:, :2]) @ W_hp
  leaving only a rank-2 exogenous input per step: [1 (bias); spd_t].
  All sigmoids/tanhs are computed as sigmoids via tanh(x) = 2*sigmoid(2x) - 1,
  with the 2x/0.5x factors pre-folded into weights; the device keeps
  hhat = h/2 as its hidden state.

Precision/speed: fp32 matmuls run at 1/4 rate on trn2 PE, so every matmul
uses a bf16 hi+lo pair decomposition (x = xh + xl, W = Wh + Wl):
  W x ~= Wh xh + Wh xl + Wl xh    (error ~2^-18, fp32 PSUM accumulation)
Weights are pre-split on the host; the hidden state is split on-device each
step (hh = bf16(h), hl = bf16(h - hh)). The cell state c stays full fp32.

Device layout (per core, batch 4096 of 32768):
  states [128 hidden partitions, batch free]; gates via PE matmuls into PSUM
  (4 banks/chunk, double buffered); one fused sigmoid ACT op per chunk over
  all 4 gates; cell update via scalar_tensor_tensor ops on DVE/GPSIMD;
  position readout via transposed matmuls (hhat stationary, [W_hp_hi|W_hp_lo]
  moving) packed into a [128, 128] PSUM tile, pair-summed on the host.
"""

import numpy as np

B, E, H, T = 32768, 64, 128, 12
NCORES = 8
BC = B // NCORES          # 4096 batch per core
NG = 4                    # elementwise groups per step
GW = BC // NG             # 1024 group width
CW = 512                  # matmul chunk width (PSUM bank = 512 fp32)
NCHUNK = BC // CW         # 8
NK = BC // 128            # 32 rel chunks

_STATE = {}


def _import_concourse():
    import sys
    for p in ('/opt/trn_rl_repo', '/root/.axon_site/_ro/trn_rl_repo'):
        if p not in sys.path:
            sys.path.append(p)


def _bf16(x):
    import ml_dtypes
    return np.asarray(x, np.float32).astype(ml_dtypes.bfloat16)


def _pair(x):
    """Split fp32 into (hi, lo) bf16 pair with hi + lo ~= x."""
    hi = _bf16(x)
    lo = _bf16(np.asarray(x, np.float32) - hi.astype(np.float32))
    return hi, lo


def _pack3(vh, vl):
    """rhs rows [vh, vl, vh] matching lhsT rows [wh, wh, wl]."""
    return [vh, vl, vh]


def _fold_weights(W_se, b_se, W_ih, W_hh, b_ih, b_hh, W_hp, b_hp):
    """Host-side weight folding in float64; returns device weight arrays."""
    f8 = np.float64
    W_se, b_se, W_ih, W_hh = W_se.astype(f8), b_se.astype(f8), W_ih.astype(f8), W_hh.astype(f8)
    b_ih, b_hh, W_hp, b_hp = b_ih.astype(f8), b_hh.astype(f8), W_hp.astype(f8), b_hp.astype(f8)

    WU = W_ih @ W_se                  # [4H, 3]
    R = WU[:, 0:2]
    w_spd = WU[:, 2]
    b0 = b_ih + b_hh + W_ih @ b_se
    W_hh_eff = W_hh + R @ W_hp
    b_eff = b0 + R @ b_hp

    def reorder(w):                   # torch [i,f,g,o] -> device [i,f,o,g]
        i, f, g, o = np.split(w, 4, axis=0)
        return np.concatenate([i, f, o, g], axis=0)

    gscale = np.ones((4 * H, 1))
    gscale[3 * H:] = 2.0              # g block: tanh(x) = 2 sig(2x) - 1

    lhsT0_hh = (reorder(W_hh) * gscale * 2.0).T.astype(np.float32)       # [H, 4H]
    lhsT_hh = (reorder(W_hh_eff) * gscale * 2.0).T.astype(np.float32)    # [H, 4H]
    w00h, w00l = _pair(lhsT0_hh)
    whhh, whhl = _pair(lhsT_hh)

    gs1 = gscale[:, 0]
    bb = reorder(b0) * gs1
    rx = reorder(R[:, 0]) * gs1
    ry = reorder(R[:, 1]) * gs1
    ws = reorder(w_spd) * gs1
    be = reorder(b_eff) * gs1
    bbh, bbl = _pair(bb)
    rxh, rxl = _pair(rx)
    ryh, ryl = _pair(ry)
    wsh, wsl = _pair(ws)
    beh, bel = _pair(be)
    # step 0: rhs rows [1,1, vxh,vxl,vxh, vyh,vyl,vyh, sh,sl,sh]
    wu0p = np.stack([bbh, bbl, rxh, rxh, rxl, ryh, ryh, ryl, wsh, wsh, wsl], axis=0)  # [11, 4H] bf16
    # steps >=1: rhs rows [1,1, sh,sl,sh]
    wsbp = np.stack([beh, bel, wsh, wsh, wsl], axis=0)                                # [5, 4H]

    whp2 = (2.0 * W_hp).T.astype(np.float32)      # [H, 2]
    wph, wpl = _pair(whp2)
    whp4 = np.concatenate([wph, wpl], axis=1)     # [H, 4] bf16
    return dict(w_hh0_h=w00h, w_hh0_l=w00l, w_hh_h=whhh, w_hh_l=whhl,
                w_u0=np.ascontiguousarray(wu0p), w_sb=np.ascontiguousarray(wsbp),
                w_hp=np.ascontiguousarray(whp4))


def _build_program(repeat=1):
    _import_concourse()
    import concourse.bacc as bacc
    import concourse.tile as tile
    from concourse import mybir

    fp32 = mybir.dt.float32
    bf16 = mybir.dt.bfloat16
    AF = mybir.ActivationFunctionType
    OP = mybir.AluOpType

    nc = bacc.Bacc("TRN2", target_bir_lowering=False, debug=False)

    # DRAM I/O (per-core shapes)
    d_h0h = nc.dram_tensor("h0h", [H, BC], bf16, kind="ExternalInput")
    d_h0l = nc.dram_tensor("h0l", [H, BC], bf16, kind="ExternalInput")
    d_c0 = nc.dram_tensor("c0t", [H, BC], fp32, kind="ExternalInput")
    d_spd = nc.dram_tensor("spdp", [5 * T, BC], bf16, kind="ExternalInput")
    d_u0 = nc.dram_tensor("u0p", [11, BC], bf16, kind="ExternalInput")
    d_whh0h = nc.dram_tensor("w_hh0_h", [H, 4 * H], bf16, kind="ExternalInput")
    d_whh0l = nc.dram_tensor("w_hh0_l", [H, 4 * H], bf16, kind="ExternalInput")
    d_whhh = nc.dram_tensor("w_hh_h", [H, 4 * H], bf16, kind="ExternalInput")
    d_whhl = nc.dram_tensor("w_hh_l", [H, 4 * H], bf16, kind="ExternalInput")
    d_wu0 = nc.dram_tensor("w_u0", [11, 4 * H], bf16, kind="ExternalInput")
    d_wsb = nc.dram_tensor("w_sb", [5, 4 * H], bf16, kind="ExternalInput")
    d_whp = nc.dram_tensor("w_hp", [H, 4], bf16, kind="ExternalInput")
    d_rels = nc.dram_tensor("rels", [T, 128, 4 * NK], fp32, kind="ExternalOutput")
    d_hout = nc.dram_tensor("hout", [2, H, BC], bf16, kind="ExternalOutput")

    with tile.TileContext(nc) as tc:
        with (
            tc.tile_pool(name="singles", bufs=1) as singles,
            tc.tile_pool(name="tmps", bufs=2) as tmps,
            tc.tile_pool(name="relsb", bufs=2) as relsb,
            tc.tile_pool(name="psum", bufs=2, space="PSUM") as psum,
        ):
            # --- load weights/constants ---
            whh0h = singles.tile([H, 4 * H], bf16)
            whh0l = singles.tile([H, 4 * H], bf16)
            whhh = singles.tile([H, 4 * H], bf16)
            whhl = singles.tile([H, 4 * H], bf16)
            wu0 = singles.tile([11, 4 * H], bf16)
            wsb = singles.tile([5, 4 * H], bf16)
            whp = singles.tile([H, 4], bf16)
            u0 = singles.tile([11, BC], bf16)
            for dst, src in ((whh0h, d_whh0h), (whh0l, d_whh0l), (whhh, d_whhh),
                             (whhl, d_whhl), (wu0, d_wu0), (wsb, d_wsb),
                             (whp, d_whp), (u0, d_u0)):
                nc.sync.dma_start(dst[:], src[:])

            # --- state tiles ---
            HhH = [[singles.tile([H, 2, CW], bf16, tag=f"hh{p}g{g}", name=f"hh{p}g{g}")
                    for g in range(NG)] for p in range(2)]
            HhL = [[singles.tile([H, 2, CW], bf16, tag=f"hl{p}g{g}", name=f"hl{p}g{g}")
                    for g in range(NG)] for p in range(2)]
            C = [singles.tile([H, 2, CW], fp32, tag=f"c{g}", name=f"c{g}") for g in range(NG)]
            SIG = [singles.tile([H, 2, 4, CW], fp32, tag=f"sig{g}", name=f"sig{g}") for g in range(NG)]
            for g in range(NG):
                sl = slice(g * GW, (g + 1) * GW)
                nc.sync.dma_start(HhH[0][g][:], d_h0h[:, sl].rearrange("h (a b) -> h a b", b=CW))
                nc.sync.dma_start(HhL[0][g][:], d_h0l[:, sl].rearrange("h (a b) -> h a b", b=CW))
                nc.sync.dma_start(C[g][:], d_c0[:, sl].rearrange("h (a b) -> h a b", b=CW))

            # --- recurrent steps ---
            for t in range(T * repeat):
                t = t % T
                par = t % 2
                HcH, HcL = HhH[par], HhL[par]
                HnH, HnL = HhH[1 - par], HhL[1 - par]
                whA, whB = (whh0h, whh0l) if t == 0 else (whhh, whhl)
                if t > 0:
                    spd_t = tmps.tile([5, BC], bf16, tag="spd", name=f"spd{t}")
                    nc.sync.dma_start(spd_t[:], d_spd[5 * t:5 * t + 5, :])

                for ch in range(NCHUNK):
                    g, j = ch // 2, ch % 2
                    cs = slice(ch * CW, (ch + 1) * CW)
                    rh = HcH[g][:, j, :]
                    rl = HcL[g][:, j, :]
                    if t == 0:
                        rhs_u, wu = u0[:, cs], wu0
                    else:
                        rhs_u, wu = spd_t[:, cs], wsb
                    gt = psum.tile([128, 4, CW], fp32, tag="gates")
                    for gi in range(4):
                        ws = slice(gi * 128, (gi + 1) * 128)
                        nc.tensor.matmul(gt[:, gi, :], whA[:, ws], rh, start=True, stop=False)
                        nc.tensor.matmul(gt[:, gi, :], whA[:, ws], rl, start=False, stop=False)
                        nc.tensor.matmul(gt[:, gi, :], whB[:, ws], rh, start=False, stop=False)
                        nc.tensor.matmul(gt[:, gi, :], wu[:, ws], rhs_u, start=False, stop=True)
                    nc.scalar.activation(SIG[g][:, j, :, :], gt[:, :, :], AF.Sigmoid)

                # cell update per group of 1024
                for g in range(NG):
                    si = SIG[g][:, :, 0, :]
                    sf = SIG[g][:, :, 1, :]
                    so = SIG[g][:, :, 2, :]
                    sg2 = SIG[g][:, :, 3, :]
                    u_t = tmps.tile([H, 2, CW], fp32, tag="u")
                    v_t = tmps.tile([H, 2, CW], fp32, tag="v")
                    w2c = tmps.tile([H, 2, CW], fp32, tag="w")
                    hf = tmps.tile([H, 2, CW], fp32, tag="hf")
                    # u = (sig(2g) - 0.5) * sig(i)   [= tanh(g)*sig(i)/2]
                    nc.vector.scalar_tensor_tensor(u_t[:], sg2, 0.5, si,
                                                   op0=OP.subtract, op1=OP.mult)
                    # v = sig(f) * c   (gpsimd keeps DVE free)
                    nc.gpsimd.tensor_tensor(v_t[:], sf, C[g][:], op=OP.mult)
                    # c' = 2u + v
                    nc.vector.scalar_tensor_tensor(C[g][:], u_t[:], 2.0, v_t[:],
                                                   op0=OP.mult, op1=OP.add)
                    # w = sig(2 c')
                    nc.scalar.activation(w2c[:], C[g][:], AF.Sigmoid, scale=2.0)
                    # hhat' = (w - 0.5) * sig(o)   [= tanh(c')*sig(o)/2]
                    nc.vector.scalar_tensor_tensor(hf[:], w2c[:], 0.5, so,
                                                   op0=OP.subtract, op1=OP.mult)
                    # bf16 pair split of the new hidden state
                    nc.gpsimd.tensor_copy(HnH[g][:], hf[:])
                    nc.vector.tensor_tensor(HnL[g][:], hf[:], HnH[g][:], op=OP.subtract)

                # rel readout: rel_t = hhat_{t+1}^T @ [wph | wpl], host sums the col pairs
                rp = psum.tile([128, 4 * NK], fp32, tag="gates")
                for k in range(NK):
                    g, r = k // (GW // 128), k % (GW // 128)
                    j, kk = r // (CW // 128), r % (CW // 128)
                    ks = slice(kk * 128, (kk + 1) * 128)
                    nc.tensor.matmul(rp[:, 4 * k:4 * k + 4], HnH[g][:, j, ks], whp[:],
                                     start=True, stop=False)
                    nc.tensor.matmul(rp[:, 4 * k:4 * k + 4], HnL[g][:, j, ks], whp[:],
                                     start=False, stop=True)
                rsb = relsb.tile([128, 4 * NK], fp32, tag="rsb")
                nc.scalar.copy(rsb[:], rp[:])
                nc.sync.dma_start(d_rels[t], rsb[:])

            # final hidden state out (bf16 pair of hhat; host sums and doubles)
            for g in range(NG):
                sl = slice(g * GW, (g + 1) * GW)
                nc.sync.dma_start(d_hout[0, :, sl].rearrange("h (a b) -> h a b", b=CW),
                                  HhH[T % 2][g][:])
                nc.sync.dma_start(d_hout[1, :, sl].rearrange("h (a b) -> h a b", b=CW),
                                  HhL[T % 2][g][:])

    nc.compile()
    return nc


def _build_runner(nc):
    """Cached jitted SPMD runner (mirrors bass2jax.run_bass_via_pjrt)."""
    _import_concourse()
    import jax
    from jax.sharding import Mesh, PartitionSpec
    from jax.experimental.shard_map import shard_map
    from concourse import bass2jax, mybir

    bass2jax.install_neuronx_cc_hook()

    partition_name = nc.partition_id_tensor.name if nc.partition_id_tensor else None

    in_names, out_names, out_avals, zero_shapes = [], [], [], []
    for alloc in nc.m.functions[0].allocations:
        if not isinstance(alloc, mybir.MemoryLocationSet):
            continue
        name = alloc.memorylocations[0].name
        if alloc.kind == "ExternalInput":
            if name != partition_name:
                in_names.append(name)
        elif alloc.kind == "ExternalOutput":
            out_names.append(name)
            shape = tuple(alloc.tensor_shape)
            dtype = mybir.dt.np(alloc.dtype)
            out_avals.append(jax.core.ShapedArray(shape, dtype))
            zero_shapes.append((shape, dtype))
    n_params = len(in_names)
    n_outs = len(out_names)
    all_names = in_names + out_names
    if partition_name is not None:
        all_names = all_names + [partition_name]
    donate = tuple(range(n_params, n_params + n_outs))

    def _body(*args):
        operands = list(args)
        if partition_name is not None:
            operands.append(bass2jax.partition_id_tensor())
        outs = bass2jax._bass_exec_p.bind(
            *operands,
            out_avals=tuple(out_avals),
            in_names=tuple(all_names),
            out_names=tuple(out_names),
            lowering_input_output_aliases=(),
            sim_require_finite=False,
            sim_require_nnan=False,
            nc=nc,
        )
        return tuple(outs)

    devices = jax.devices()[:NCORES]
    mesh = Mesh(np.asarray(devices), ("core",))
    in_specs = (PartitionSpec("core"),) * (n_params + n_outs)
    out_specs = (PartitionSpec("core"),) * n_outs
    sharded = jax.jit(
        shard_map(_body, mesh=mesh, in_specs=in_specs, out_specs=out_specs,
                  check_rep=False),
        donate_argnums=donate, keep_unused=True,
    )

    def run(in_maps):
        concat_in = [np.concatenate([m[name] for m in in_maps], axis=0)
                     for name in in_names]
        concat_zeros = [np.zeros((NCORES * s[0], *s[1:]), d) for s, d in zero_shapes]
        out_arrs = sharded(*concat_in, *concat_zeros)
        return {
            name: np.asarray(out_arrs[i]).reshape(NCORES, *zero_shapes[i][0])
            for i, name in enumerate(out_names)
        }

    return run


def _get_runner():
    if "run" not in _STATE:
        nc = _build_program()
        _STATE["nc"] = nc
        _STATE["run"] = _build_runner(nc)
    return _STATE["run"]


def measure_device_time(in_maps, iters=20, warmup=2):
    """Steady-state per-execution time with device-resident inputs."""
    _import_concourse()
    import time
    import jax
    from jax.sharding import Mesh, PartitionSpec, NamedSharding
    from jax.experimental.shard_map import shard_map
    from concourse import bass2jax, mybir

    _get_runner()
    nc = _STATE["nc"]
    partition_name = nc.partition_id_tensor.name if nc.partition_id_tensor else None
    in_names, out_names, out_avals, zero_shapes = [], [], [], []
    for alloc in nc.m.functions[0].allocations:
        if not isinstance(alloc, mybir.MemoryLocationSet):
            continue
        name = alloc.memorylocations[0].name
        if alloc.kind == "ExternalInput":
            if name != partition_name:
                in_names.append(name)
        elif alloc.kind == "ExternalOutput":
            out_names.append(name)
            shape = tuple(alloc.tensor_shape)
            dtype = mybir.dt.np(alloc.dtype)
            out_avals.append(jax.core.ShapedArray(shape, dtype))
            zero_shapes.append((shape, dtype))
    n_params = len(in_names)
    all_names = in_names + out_names
    if partition_name is not None:
        all_names = all_names + [partition_name]

    def _tbody(*ins):
        operands = list(ins)
        if partition_name is not None:
            operands.append(bass2jax.partition_id_tensor())
        return tuple(bass2jax._bass_exec_p.bind(
            *operands,
            out_avals=tuple(out_avals),
            in_names=tuple(all_names),
            out_names=tuple(out_names),
            lowering_input_output_aliases=(),
            sim_require_finite=False,
            sim_require_nnan=False,
            nc=nc,
        ))

    devices = jax.devices()[:NCORES]
    mesh = Mesh(np.asarray(devices), ("core",))
    P = PartitionSpec
    n_outs = len(out_names)
    tfn = jax.jit(shard_map(_tbody, mesh=mesh,
                            in_specs=(P("core"),) * (n_params + n_outs),
                            out_specs=(P("core"),) * n_outs, check_rep=False))

    sh = NamedSharding(mesh, P("core"))
    dev_in = [jax.device_put(np.concatenate([m[n] for m in in_maps], axis=0), sh)
              for n in in_names]
    dev_in += [jax.device_put(np.zeros((NCORES * s[0], *s[1:]), d), sh)
               for s, d in zero_shapes]
    for _ in range(warmup):
        jax.block_until_ready(tfn(*dev_in))
    t0 = time.perf_counter()
    rs = [tfn(*dev_in) for _ in range(iters)]
    jax.block_until_ready(rs)
    dt = (time.perf_counter() - t0) / iters
    stimes = []
    for _ in range(max(1, iters // 2)):
        t0 = time.perf_counter()
        jax.block_until_ready(tfn(*dev_in))
        stimes.append(time.perf_counter() - t0)
    return dt, min(stimes)


def _make_in_maps(last_pos_rel, h0, c0, pred_ped_speed, wd):
    h0t = np.ascontiguousarray(h0[0].T).astype(np.float32) * 0.5   # hhat
    h0h, h0l = _pair(h0t)
    c0t = np.ascontiguousarray(c0[0].T).astype(np.float32)
    relT = last_pos_rel.T.astype(np.float32)
    spd = pred_ped_speed[:, :, 0].astype(np.float32)               # [T, B]

    sh, slo = _pair(spd)
    ones = np.ones((B,), np.float32)
    oneb = _bf16(ones)
    # spdp rows per t: [1, 1, sh, sl, sh]
    spdp = np.empty((5 * T, B), dtype=oneb.dtype)
    for t in range(T):
        spdp[5 * t + 0] = oneb
        spdp[5 * t + 1] = oneb
        spdp[5 * t + 2] = sh[t]
        spdp[5 * t + 3] = slo[t]
        spdp[5 * t + 4] = sh[t]
    rxh, rxl = _pair(relT[0])
    ryh, ryl = _pair(relT[1])
    u0p = np.stack([oneb, oneb, rxh, rxl, rxh, ryh, ryl, ryh,
                    sh[0], slo[0], sh[0]], axis=0)

    in_maps = []
    for c in range(NCORES):
        s = slice(c * BC, (c + 1) * BC)
        m = dict(h0h=np.ascontiguousarray(h0h[:, s]),
                 h0l=np.ascontiguousarray(h0l[:, s]),
                 c0t=np.ascontiguousarray(c0t[:, s]),
                 spdp=np.ascontiguousarray(spdp[:, s]),
                 u0p=np.ascontiguousarray(u0p[:, s]))
        m.update(wd)
        in_maps.append(m)
    return in_maps


def kernel(last_pos, last_pos_rel, h0, c0, pred_ped_speed,
           W_se, b_se, W_ih, W_hh, b_ih, b_hh, W_hp, b_hp):
    last_pos_rel = np.asarray(last_pos_rel, np.float32)
    h0 = np.asarray(h0, np.float32)
    c0 = np.asarray(c0, np.float32)
    pred_ped_speed = np.asarray(pred_ped_speed, np.float32)

    wd = _fold_weights(np.asarray(W_se), np.asarray(b_se), np.asarray(W_ih),
                       np.asarray(W_hh), np.asarray(b_ih), np.asarray(b_hh),
                       np.asarray(W_hp), np.asarray(b_hp))
    in_maps = _make_in_maps(last_pos_rel, h0, c0, pred_ped_speed, wd)

    run = _get_runner()
    outs = run(in_maps)

    # rels: [core][T, 128, 4*NK]; rel = cols[0:2] + cols[2:4] per k; + b_hp
    r = outs["rels"].reshape(NCORES, T, 128, NK, 2, 2)
    r = r[..., 0, :] + r[..., 1, :]                    # [NCORES, T, 128, NK, 2]
    r = r.transpose(1, 0, 3, 2, 4).reshape(T, B, 2)
    rels = (r + np.asarray(b_hp, np.float32)[None, None, :]).astype(np.float32)

    h = outs["hout"].astype(np.float32)                # [NCORES, 2, H, BC]
    h = 2.0 * (h[:, 0] + h[:, 1])                      # [NCORES, H, BC]
    h = h.transpose(0, 2, 1).reshape(1, B, H).astype(np.float32)
    return np.ascontiguousarray(rels), np.ascontiguousarray(h)


# revision 32
# speedup vs baseline: 6185.7685x; 1.5117x over previous
"""Trainium2 Bass kernel for the LSTM trajectory decoder.

Algorithm notes (host-side algebraic folding):
  The reference per-step computation is
      x_t   = W_se @ [rel_{t-1}; spd_t] + b_se          (spatial embedding)
      gates = W_ih @ x_t + W_hh @ h_t + b
      c,h   = LSTM cell update
      rel_t = W_hp @ h_{t+1} + b_hp
  Since rel_{t-1} = W_hp @ h_t + b_hp is linear in h_t, the whole embedding
  path folds into the recurrent weights:
      W_hh_eff = W_hh + (W_ih @ W_se[:, :2]) @ W_hp
  leaving only a rank-2 exogenous input per step: [1 (bias); spd_t].
  All sigmoids/tanhs are computed as sigmoids via tanh(x) = 2*sigmoid(2x) - 1,
  with the 2x/0.5x factors pre-folded into weights; the device keeps
  hhat = h/2 as its hidden state.

Precision/speed: fp32 matmuls run at 1/4 rate on trn2 PE, so every matmul
uses a bf16 hi+lo pair decomposition (x = xh + xl, W = Wh + Wl):
  W x ~= Wh xh + Wh xl + Wl xh    (error ~2^-18, fp32 PSUM accumulation)
Weights are pre-split on the host; the hidden state is split on-device each
step (hh = bf16(h), hl = bf16(h - hh)). The cell state c stays full fp32.

Device layout (per core, batch 4096 of 32768):
  states [128 hidden partitions, batch free]; gates via PE matmuls into PSUM
  (4 banks/chunk, double buffered); one fused sigmoid ACT op per chunk over
  all 4 gates; cell update via scalar_tensor_tensor ops on DVE/GPSIMD;
  position readout via transposed matmuls (hhat stationary, [W_hp_hi|W_hp_lo]
  moving) packed into a [128, 128] PSUM tile, pair-summed on the host.
"""

import numpy as np

B, E, H, T = 32768, 64, 128, 12
NCORES = 8
BC = B // NCORES          # 4096 batch per core
NG = 4                    # elementwise groups per step
GW = BC // NG             # 1024 group width
CW = 512                  # matmul chunk width (PSUM bank = 512 fp32)
NCHUNK = BC // CW         # 8
NK = BC // 128            # 32 rel chunks

_STATE = {}


def _import_concourse():
    import sys
    for p in ('/opt/trn_rl_repo', '/root/.axon_site/_ro/trn_rl_repo'):
        if p not in sys.path:
            sys.path.append(p)


def _bf16(x):
    import ml_dtypes
    return np.asarray(x, np.float32).astype(ml_dtypes.bfloat16)


def _pair(x):
    """Split fp32 into (hi, lo) bf16 pair with hi + lo ~= x."""
    hi = _bf16(x)
    lo = _bf16(np.asarray(x, np.float32) - hi.astype(np.float32))
    return hi, lo


def _pack3(vh, vl):
    """rhs rows [vh, vl, vh] matching lhsT rows [wh, wh, wl]."""
    return [vh, vl, vh]


def _fold_weights(W_se, b_se, W_ih, W_hh, b_ih, b_hh, W_hp, b_hp):
    """Host-side weight folding in float64; returns device weight arrays."""
    f8 = np.float64
    W_se, b_se, W_ih, W_hh = W_se.astype(f8), b_se.astype(f8), W_ih.astype(f8), W_hh.astype(f8)
    b_ih, b_hh, W_hp, b_hp = b_ih.astype(f8), b_hh.astype(f8), W_hp.astype(f8), b_hp.astype(f8)

    WU = W_ih @ W_se                  # [4H, 3]
    R = WU[:, 0:2]
    w_spd = WU[:, 2]
    b0 = b_ih + b_hh + W_ih @ b_se
    W_hh_eff = W_hh + R @ W_hp
    b_eff = b0 + R @ b_hp

    def reorder(w):                   # torch [i,f,g,o] -> device [i,f,o,g]
        i, f, g, o = np.split(w, 4, axis=0)
        return np.concatenate([i, f, o, g], axis=0)

    gscale = np.ones((4 * H, 1))
    gscale[3 * H:] = 2.0              # g block: tanh(x) = 2 sig(2x) - 1

    lhsT0_hh = (reorder(W_hh) * gscale * 2.0).T.astype(np.float32)       # [H, 4H]
    lhsT_hh = (reorder(W_hh_eff) * gscale * 2.0).T.astype(np.float32)    # [H, 4H]
    w00h, w00l = _pair(lhsT0_hh)
    whhh, whhl = _pair(lhsT_hh)

    gs1 = gscale[:, 0]
    bb = reorder(b0) * gs1
    rx = reorder(R[:, 0]) * gs1
    ry = reorder(R[:, 1]) * gs1
    ws = reorder(w_spd) * gs1
    be = reorder(b_eff) * gs1
    bbh, bbl = _pair(bb)
    rxh, rxl = _pair(rx)
    ryh, ryl = _pair(ry)
    wsh, wsl = _pair(ws)
    beh, bel = _pair(be)
    # step 0: rhs rows [1,1, vxh,vxl,vxh, vyh,vyl,vyh, sh,sl,sh]
    wu0p = np.stack([bbh, bbl, rxh, rxh, rxl, ryh, ryh, ryl, wsh, wsh, wsl], axis=0)  # [11, 4H] bf16
    # steps >=1: rhs rows [1,1, sh,sl,sh]
    wsbp = np.stack([beh, bel, wsh, wsh, wsl], axis=0)                                # [5, 4H]

    whp2 = (2.0 * W_hp).T.astype(np.float32)      # [H, 2]
    wph, wpl = _pair(whp2)
    whp4 = np.concatenate([wph, wpl], axis=1)     # [H, 4] bf16
    return dict(w_hh0_h=w00h, w_hh0_l=w00l, w_hh_h=whhh, w_hh_l=whhl,
                w_u0=np.ascontiguousarray(wu0p), w_sb=np.ascontiguousarray(wsbp),
                w_hp=np.ascontiguousarray(whp4))


def _build_program(repeat=1, rel_mode="ws"):
    # rel_mode: "ws" = W_hp stationary (one weight load, 16 full streams),
    #           "tr" = transposed (hhat stationary, 64 small matmuls),
    #           "off" = skip readout (timing probe only)
    _import_concourse()
    import concourse.bacc as bacc
    import concourse.tile as tile
    from concourse import mybir

    fp32 = mybir.dt.float32
    bf16 = mybir.dt.bfloat16
    AF = mybir.ActivationFunctionType
    OP = mybir.AluOpType

    nc = bacc.Bacc("TRN2", target_bir_lowering=False, debug=False)

    # DRAM I/O (per-core shapes)
    d_h0h = nc.dram_tensor("h0h", [H, BC], bf16, kind="ExternalInput")
    d_h0l = nc.dram_tensor("h0l", [H, BC], bf16, kind="ExternalInput")
    d_c0 = nc.dram_tensor("c0t", [H, BC], fp32, kind="ExternalInput")
    d_spd = nc.dram_tensor("spdp", [5 * T, BC], bf16, kind="ExternalInput")
    d_u0 = nc.dram_tensor("u0p", [11, BC], bf16, kind="ExternalInput")
    d_whh0h = nc.dram_tensor("w_hh0_h", [H, 4 * H], bf16, kind="ExternalInput")
    d_whh0l = nc.dram_tensor("w_hh0_l", [H, 4 * H], bf16, kind="ExternalInput")
    d_whhh = nc.dram_tensor("w_hh_h", [H, 4 * H], bf16, kind="ExternalInput")
    d_whhl = nc.dram_tensor("w_hh_l", [H, 4 * H], bf16, kind="ExternalInput")
    d_wu0 = nc.dram_tensor("w_u0", [11, 4 * H], bf16, kind="ExternalInput")
    d_wsb = nc.dram_tensor("w_sb", [5, 4 * H], bf16, kind="ExternalInput")
    d_whp = nc.dram_tensor("w_hp", [H, 4], bf16, kind="ExternalInput")
    rels_shape = [T, 128, 4 * NK] if rel_mode == "tr" else [T, 4, BC]
    d_rels = nc.dram_tensor("rels", rels_shape, fp32, kind="ExternalOutput")
    d_hout = nc.dram_tensor("hout", [2, H, BC], bf16, kind="ExternalOutput")

    with tile.TileContext(nc) as tc:
        with (
            tc.tile_pool(name="singles", bufs=1) as singles,
            tc.tile_pool(name="tmps", bufs=2) as tmps,
            tc.tile_pool(name="relsb", bufs=2) as relsb,
            tc.tile_pool(name="psum", bufs=2, space="PSUM") as psum,
        ):
            # --- load weights/constants ---
            whh0h = singles.tile([H, 4 * H], bf16)
            whh0l = singles.tile([H, 4 * H], bf16)
            whhh = singles.tile([H, 4 * H], bf16)
            whhl = singles.tile([H, 4 * H], bf16)
            wu0 = singles.tile([11, 4 * H], bf16)
            wsb = singles.tile([5, 4 * H], bf16)
            whp = singles.tile([H, 4], bf16)
            u0 = singles.tile([11, BC], bf16)
            for dst, src in ((whh0h, d_whh0h), (whh0l, d_whh0l), (whhh, d_whhh),
                             (whhl, d_whhl), (wu0, d_wu0), (wsb, d_wsb),
                             (whp, d_whp), (u0, d_u0)):
                nc.sync.dma_start(dst[:], src[:])

            # --- state tiles ---
            HhH = [[singles.tile([H, 2, CW], bf16, tag=f"hh{p}g{g}", name=f"hh{p}g{g}")
                    for g in range(NG)] for p in range(2)]
            HhL = [[singles.tile([H, 2, CW], bf16, tag=f"hl{p}g{g}", name=f"hl{p}g{g}")
                    for g in range(NG)] for p in range(2)]
            C = [singles.tile([H, 2, CW], fp32, tag=f"c{g}", name=f"c{g}") for g in range(NG)]
            SIG = [singles.tile([H, 2, 4, CW], fp32, tag=f"sig{g}", name=f"sig{g}") for g in range(NG)]
            for g in range(NG):
                sl = slice(g * GW, (g + 1) * GW)
                nc.sync.dma_start(HhH[0][g][:], d_h0h[:, sl].rearrange("h (a b) -> h a b", b=CW))
                nc.sync.dma_start(HhL[0][g][:], d_h0l[:, sl].rearrange("h (a b) -> h a b", b=CW))
                nc.sync.dma_start(C[g][:], d_c0[:, sl].rearrange("h (a b) -> h a b", b=CW))

            # --- recurrent steps ---
            for t in range(T * repeat):
                t = t % T
                par = t % 2
                HcH, HcL = HhH[par], HhL[par]
                HnH, HnL = HhH[1 - par], HhL[1 - par]
                whA, whB = (whh0h, whh0l) if t == 0 else (whhh, whhl)
                if t > 0:
                    spd_t = tmps.tile([5, BC], bf16, tag="spd", name=f"spd{t}")
                    nc.sync.dma_start(spd_t[:], d_spd[5 * t:5 * t + 5, :])

                for ch in range(NCHUNK):
                    g, j = ch // 2, ch % 2
                    cs = slice(ch * CW, (ch + 1) * CW)
                    rh = HcH[g][:, j, :]
                    rl = HcL[g][:, j, :]
                    if t == 0:
                        rhs_u, wu = u0[:, cs], wu0
                    else:
                        rhs_u, wu = spd_t[:, cs], wsb
                    gt = psum.tile([128, 4, CW], fp32, tag="gates")
                    for gi in range(4):
                        ws = slice(gi * 128, (gi + 1) * 128)
                        nc.tensor.matmul(gt[:, gi, :], whA[:, ws], rh, start=True, stop=False)
                        nc.tensor.matmul(gt[:, gi, :], whA[:, ws], rl, start=False, stop=False)
                        nc.tensor.matmul(gt[:, gi, :], whB[:, ws], rh, start=False, stop=False)
                        nc.tensor.matmul(gt[:, gi, :], wu[:, ws], rhs_u, start=False, stop=True)
                    nc.scalar.activation(SIG[g][:, j, :, :], gt[:, :, :], AF.Sigmoid)

                # cell update per group of 1024
                for g in range(NG):
                    si = SIG[g][:, :, 0, :]
                    sf = SIG[g][:, :, 1, :]
                    so = SIG[g][:, :, 2, :]
                    sg2 = SIG[g][:, :, 3, :]
                    u_t = tmps.tile([H, 2, CW], fp32, tag="u")
                    v_t = tmps.tile([H, 2, CW], fp32, tag="v")
                    w2c = tmps.tile([H, 2, CW], fp32, tag="w")
                    hf = tmps.tile([H, 2, CW], fp32, tag="hf")
                    # u = (sig(2g) - 0.5) * sig(i)   [= tanh(g)*sig(i)/2]
                    nc.vector.scalar_tensor_tensor(u_t[:], sg2, 0.5, si,
                                                   op0=OP.subtract, op1=OP.mult)
                    # v = sig(f) * c   (gpsimd keeps DVE free)
                    nc.gpsimd.tensor_tensor(v_t[:], sf, C[g][:], op=OP.mult)
                    # c' = 2u + v
                    nc.vector.scalar_tensor_tensor(C[g][:], u_t[:], 2.0, v_t[:],
                                                   op0=OP.mult, op1=OP.add)
                    # w = sig(2 c')
                    nc.scalar.activation(w2c[:], C[g][:], AF.Sigmoid, scale=2.0)
                    # hhat' = (w - 0.5) * sig(o)   [= tanh(c')*sig(o)/2]
                    nc.vector.scalar_tensor_tensor(hf[:], w2c[:], 0.5, so,
                                                   op0=OP.subtract, op1=OP.mult)
                    # bf16 pair split of the new hidden state
                    nc.gpsimd.tensor_copy(HnH[g][:], hf[:])
                    nc.vector.tensor_tensor(HnL[g][:], hf[:], HnH[g][:], op=OP.subtract)

                # rel readout: rel_raw = [wph | wpl]^T @ hhat_{t+1}
                if rel_mode == "off":
                    continue
                if rel_mode == "tr":
                    # hhat chunks stationary, whp moving (64 small matmuls)
                    rp = psum.tile([128, 4 * NK], fp32, tag="gates")
                    for k in range(NK):
                        g, r = k // (GW // 128), k % (GW // 128)
                        j, kk = r // (CW // 128), r % (CW // 128)
                        ks = slice(kk * 128, (kk + 1) * 128)
                        nc.tensor.matmul(rp[:, 4 * k:4 * k + 4], HnH[g][:, j, ks], whp[:],
                                         start=True, stop=False)
                        nc.tensor.matmul(rp[:, 4 * k:4 * k + 4], HnL[g][:, j, ks], whp[:],
                                         start=False, stop=True)
                    rsb = relsb.tile([128, 4 * NK], fp32, tag="rsb")
                    nc.scalar.copy(rsb[:], rp[:])
                    nc.sync.dma_start(d_rels[t], rsb[:])
                else:
                    # whp stationary (loaded once), hhat pair moving: out rows
                    # [0:2] = rel via wph, [2:4] = rel via wpl; host sums.
                    rpA = psum.tile([4, NCHUNK // 2, CW], fp32, tag="gates", name=f"rpA{t}")
                    rpB = psum.tile([4, NCHUNK // 2, CW], fp32, tag="gates", name=f"rpB{t}")
                    for ch in range(NCHUNK):
                        g, j = ch // 2, ch % 2
                        rp = rpA if ch < 4 else rpB
                        nc.tensor.matmul(rp[:, ch % 4, :], whp[:], HnH[g][:, j, :],
                                         start=True, stop=False)
                        nc.tensor.matmul(rp[:, ch % 4, :], whp[:], HnL[g][:, j, :],
                                         start=False, stop=True)
                    rsb = relsb.tile([4, BC], fp32, tag="rsb")
                    nc.scalar.copy(rsb[:, 0:BC // 2].rearrange("p (a b) -> p a b", b=CW), rpA[:])
                    nc.vector.tensor_copy(rsb[:, BC // 2:].rearrange("p (a b) -> p a b", b=CW), rpB[:])
                    nc.sync.dma_start(d_rels[t], rsb[:])

            # final hidden state out (bf16 pair of hhat; host sums and doubles)
            for g in range(NG):
                sl = slice(g * GW, (g + 1) * GW)
                nc.sync.dma_start(d_hout[0, :, sl].rearrange("h (a b) -> h a b", b=CW),
                                  HhH[T % 2][g][:])
                nc.sync.dma_start(d_hout[1, :, sl].rearrange("h (a b) -> h a b", b=CW),
                                  HhL[T % 2][g][:])

    nc.compile()
    return nc


def _build_runner(nc):
    """Cached jitted SPMD runner (mirrors bass2jax.run_bass_via_pjrt)."""
    _import_concourse()
    import jax
    from jax.sharding import Mesh, PartitionSpec
    from jax.experimental.shard_map import shard_map
    from concourse import bass2jax, mybir

    bass2jax.install_neuronx_cc_hook()

    partition_name = nc.partition_id_tensor.name if nc.partition_id_tensor else None

    in_names, out_names, out_avals, zero_shapes = [], [], [], []
    for alloc in nc.m.functions[0].allocations:
        if not isinstance(alloc, mybir.MemoryLocationSet):
            continue
        name = alloc.memorylocations[0].name
        if alloc.kind == "ExternalInput":
            if name != partition_name:
                in_names.append(name)
        elif alloc.kind == "ExternalOutput":
            out_names.append(name)
            shape = tuple(alloc.tensor_shape)
            dtype = mybir.dt.np(alloc.dtype)
            out_avals.append(jax.core.ShapedArray(shape, dtype))
            zero_shapes.append((shape, dtype))
    n_params = len(in_names)
    n_outs = len(out_names)
    all_names = in_names + out_names
    if partition_name is not None:
        all_names = all_names + [partition_name]
    donate = tuple(range(n_params, n_params + n_outs))

    def _body(*args):
        operands = list(args)
        if partition_name is not None:
            operands.append(bass2jax.partition_id_tensor())
        outs = bass2jax._bass_exec_p.bind(
            *operands,
            out_avals=tuple(out_avals),
            in_names=tuple(all_names),
            out_names=tuple(out_names),
            lowering_input_output_aliases=(),
            sim_require_finite=False,
            sim_require_nnan=False,
            nc=nc,
        )
        return tuple(outs)

    devices = jax.devices()[:NCORES]
    mesh = Mesh(np.asarray(devices), ("core",))
    in_specs = (PartitionSpec("core"),) * (n_params + n_outs)
    out_specs = (PartitionSpec("core"),) * n_outs
    sharded = jax.jit(
        shard_map(_body, mesh=mesh, in_specs=in_specs, out_specs=out_specs,
                  check_rep=False),
        donate_argnums=donate, keep_unused=True,
    )

    def run(in_maps):
        concat_in = [np.concatenate([m[name] for m in in_maps], axis=0)
                     for name in in_names]
        concat_zeros = [np.zeros((NCORES * s[0], *s[1:]), d) for s, d in zero_shapes]
        out_arrs = sharded(*concat_in, *concat_zeros)
        return {
            name: np.asarray(out_arrs[i]).reshape(NCORES, *zero_shapes[i][0])
            for i, name in enumerate(out_names)
        }

    return run


def _get_runner():
    if "run" not in _STATE:
        nc = _build_program()
        _STATE["nc"] = nc
        _STATE["run"] = _build_runner(nc)
    return _STATE["run"]


def measure_device_time(in_maps, iters=20, warmup=2):
    """Steady-state per-execution time with device-resident inputs."""
    _import_concourse()
    import time
    import jax
    from jax.sharding import Mesh, PartitionSpec, NamedSharding
    from jax.experimental.shard_map import shard_map
    from concourse import bass2jax, mybir

    _get_runner()
    nc = _STATE["nc"]
    partition_name = nc.partition_id_tensor.name if nc.partition_id_tensor else None
    in_names, out_names, out_avals, zero_shapes = [], [], [], []
    for alloc in nc.m.functions[0].allocations:
        if not isinstance(alloc, mybir.MemoryLocationSet):
            continue
        name = alloc.memorylocations[0].name
        if alloc.kind == "ExternalInput":
            if name != partition_name:
                in_names.append(name)
        elif alloc.kind == "ExternalOutput":
            out_names.append(name)
            shape = tuple(alloc.tensor_shape)
            dtype = mybir.dt.np(alloc.dtype)
            out_avals.append(jax.core.ShapedArray(shape, dtype))
            zero_shapes.append((shape, dtype))
    n_params = len(in_names)
    all_names = in_names + out_names
    if partition_name is not None:
        all_names = all_names + [partition_name]

    def _tbody(*ins):
        operands = list(ins)
        if partition_name is not None:
            operands.append(bass2jax.partition_id_tensor())
        return tuple(bass2jax._bass_exec_p.bind(
            *operands,
            out_avals=tuple(out_avals),
            in_names=tuple(all_names),
            out_names=tuple(out_names),
            lowering_input_output_aliases=(),
            sim_require_finite=False,
            sim_require_nnan=False,
            nc=nc,
        ))

    devices = jax.devices()[:NCORES]
    mesh = Mesh(np.asarray(devices), ("core",))
    P = PartitionSpec
    n_outs = len(out_names)
    tfn = jax.jit(shard_map(_tbody, mesh=mesh,
                            in_specs=(P("core"),) * (n_params + n_outs),
                            out_specs=(P("core"),) * n_outs, check_rep=False))

    sh = NamedSharding(mesh, P("core"))
    dev_in = [jax.device_put(np.concatenate([m[n] for m in in_maps], axis=0), sh)
              for n in in_names]
    dev_in += [jax.device_put(np.zeros((NCORES * s[0], *s[1:]), d), sh)
               for s, d in zero_shapes]
    for _ in range(warmup):
        jax.block_until_ready(tfn(*dev_in))
    t0 = time.perf_counter()
    rs = [tfn(*dev_in) for _ in range(iters)]
    jax.block_until_ready(rs)
    dt = (time.perf_counter() - t0) / iters
    stimes = []
    for _ in range(max(1, iters // 2)):
        t0 = time.perf_counter()
        jax.block_until_ready(tfn(*dev_in))
        stimes.append(time.perf_counter() - t0)
    return dt, min(stimes)


def _make_in_maps(last_pos_rel, h0, c0, pred_ped_speed, wd):
    h0t = np.ascontiguousarray(h0[0].T).astype(np.float32) * 0.5   # hhat
    h0h, h0l = _pair(h0t)
    c0t = np.ascontiguousarray(c0[0].T).astype(np.float32)
    relT = last_pos_rel.T.astype(np.float32)
    spd = pred_ped_speed[:, :, 0].astype(np.float32)               # [T, B]

    sh, slo = _pair(spd)
    ones = np.ones((B,), np.float32)
    oneb = _bf16(ones)
    # spdp rows per t: [1, 1, sh, sl, sh]
    spdp = np.empty((5 * T, B), dtype=oneb.dtype)
    for t in range(T):
        spdp[5 * t + 0] = oneb
        spdp[5 * t + 1] = oneb
        spdp[5 * t + 2] = sh[t]
        spdp[5 * t + 3] = slo[t]
        spdp[5 * t + 4] = sh[t]
    rxh, rxl = _pair(relT[0])
    ryh, ryl = _pair(relT[1])
    u0p = np.stack([oneb, oneb, rxh, rxl, rxh, ryh, ryl, ryh,
                    sh[0], slo[0], sh[0]], axis=0)

    in_maps = []
    for c in range(NCORES):
        s = slice(c * BC, (c + 1) * BC)
        m = dict(h0h=np.ascontiguousarray(h0h[:, s]),
                 h0l=np.ascontiguousarray(h0l[:, s]),
                 c0t=np.ascontiguousarray(c0t[:, s]),
                 spdp=np.ascontiguousarray(spdp[:, s]),
                 u0p=np.ascontiguousarray(u0p[:, s]))
        m.update(wd)
        in_maps.append(m)
    return in_maps


def kernel(last_pos, last_pos_rel, h0, c0, pred_ped_speed,
           W_se, b_se, W_ih, W_hh, b_ih, b_hh, W_hp, b_hp):
    last_pos_rel = np.asarray(last_pos_rel, np.float32)
    h0 = np.asarray(h0, np.float32)
    c0 = np.asarray(c0, np.float32)
    pred_ped_speed = np.asarray(pred_ped_speed, np.float32)

    wd = _fold_weights(np.asarray(W_se), np.asarray(b_se), np.asarray(W_ih),
                       np.asarray(W_hh), np.asarray(b_ih), np.asarray(b_hh),
                       np.asarray(W_hp), np.asarray(b_hp))
    in_maps = _make_in_maps(last_pos_rel, h0, c0, pred_ped_speed, wd)

    run = _get_runner()
    outs = run(in_maps)

    # rels: [core][T, 4, BC]; rows [0:2] = rel via wph, [2:4] via wpl; sum + b_hp
    r = outs["rels"]                                   # [NCORES, T, 4, BC]
    r = r[:, :, 0:2, :] + r[:, :, 2:4, :]              # [NCORES, T, 2, BC]
    r = r.transpose(1, 0, 3, 2).reshape(T, B, 2)
    rels = (r + np.asarray(b_hp, np.float32)[None, None, :]).astype(np.float32)

    h = outs["hout"].astype(np.float32)                # [NCORES, 2, H, BC]
    h = 2.0 * (h[:, 0] + h[:, 1])                      # [NCORES, H, BC]
    h = h.transpose(0, 2, 1).reshape(1, B, H).astype(np.float32)
    return np.ascontiguousarray(rels), np.ascontiguousarray(h)
